# revision 23
# baseline (speedup 1.0000x reference)
"""Trainium2 Bass kernel for HGNN-MLP (email/url/sender heterograph).

Math (dead-code-eliminated vs the full module: out_url/out_sender are unused):
  out = relu( x_email @ Wer  +  T @ Wcomb + bias_row ) @ Wc + bc
where
  Wer      = W_email @ (Wroot_ue + Wroot_se)                       [768,128]
  T[d,0:9]  = sum over ue-edges into d of [x_url[src], 1]           (9 cols)
  T[d,9:11] = sum over se-edges into d of [x_sender[src], 1]        (2 cols)
  Wcomb    = [[W_url;b_url]@Wrel_ue ; [W_sender;b_sender]@Wrel_se]  [12,128]
  bias_row = brel_ue + brel_se + b_email @ (Wroot_ue + Wroot_se)

Distribution: 8-way data-parallel over destination emails. Emails are
degree-sorted on host and dealt round-robin across cores, so each 128-email
stripe holds near-equal-cost emails. Each email's ue/se edges occupy one
SBUF partition in two bf16 gather streams (9-wide url rows, 2-wide sender
rows, indirect DMA from per-type tables). Per 4-stripe quad, two strided
DVE reduces compute the segment sums (no one-hot scatter), a PE identity-
transpose per stripe flips them into the (12, emails) table consumed by the
fused projection matmuls (bf16). bias_row folds into the Activation-engine
relu; bc folds into the output copies. Outputs stream per 512-email block.
No collectives.
"""
import numpy as np
from contextlib import ExitStack

import ml_dtypes
import concourse.bacc as bacc
import concourse.mybir as mybir
from concourse.bass import IndirectOffsetOnAxis
from concourse.bass_utils import run_bass_kernel_spmd

F32 = mybir.dt.float32
BF16 = mybir.dt.bfloat16
I32 = mybir.dt.int32
BF = ml_dtypes.bfloat16

N_EMAIL, N_URL, N_SENDER = 100000, 400000, 50000
NCORE = 8
EPC = 12500                  # emails per core
NSTR = 98                    # 128-email stripes (12544 >= 12500)
EPAD = 12800                 # email cols padded for 25x512 blocks
NBLK, BW = 25, 512
NQ = 25                      # quads; quad q = stripes 4q..4q+Kq, Kq in {4,2}
ZU, ZS = N_URL, N_SENDER     # zero rows of the two gather tables
RGU = 6                      # ue gather ring (quads)
RTQ = 4                      # t_sb quad ring

_prog_cache = {}


def _Kq(q):
    return 4 if q < NQ - 1 else 2


def _build_program(layout=None):
    if layout is None:
        layout = _prog_cache["layout"]
    key = ("prog", tuple(layout["mu"]), tuple(layout["ms"]))
    if key in _prog_cache:
        return _prog_cache[key]

    mu = layout["mu"]          # ue slots per email, per quad
    ms = layout["ms"]          # se slots per email, per quad
    ou = layout["ou"]          # ue col offset per quad (len NQ+1)
    os_ = layout["os"]         # se col offset per quad
    NCOLU = int(ou[-1])
    NCOLS = int(os_[-1])
    segroups = layout["segroups"]  # list of (q0, q1) quad ranges
    sgrp_of = np.empty(NQ, np.int64)
    for gj, (q0, q1) in enumerate(segroups):
        sgrp_of[q0:q1] = gj
    WU9 = max(_Kq(q) * int(mu[q]) * 9 for q in range(NQ))  # ue ring slot elems

    nc = bacc.Bacc("TRN2")

    tabu = nc.dram_tensor("tabu", (N_URL + 1, 9), BF16, kind="ExternalInput")
    tabs = nc.dram_tensor("tabs", (N_SENDER + 1, 2), BF16, kind="ExternalInput")
    srcu = nc.dram_tensor("srcu", (128, NCOLU), I32, kind="ExternalInput")
    srcs = nc.dram_tensor("srcs", (128, NCOLS), I32, kind="ExternalInput")
    xT = nc.dram_tensor("xT", (768, EPAD), BF16, kind="ExternalInput")
    wer = nc.dram_tensor("wer", (768, 128), BF16, kind="ExternalInput")
    wcomb = nc.dram_tensor("wcomb", (12, 128), BF16, kind="ExternalInput")
    wc = nc.dram_tensor("wc", (128, 2), BF16, kind="ExternalInput")
    bc = nc.dram_tensor("bc", (2, 1), F32, kind="ExternalInput")
    zbias = nc.dram_tensor("zbias", (128, 1), F32, kind="ExternalInput")
    ident = nc.dram_tensor("ident", (128, 128), BF16, kind="ExternalInput")
    out = nc.dram_tensor("out", (2, EPAD), F32, kind="ExternalOutput")

    with ExitStack() as ctx:
        E = ctx.enter_context
        srcu_sb = E(nc.sbuf_tensor("srcu_sb", (128, NCOLU), I32))
        srcs_sb = E(nc.sbuf_tensor("srcs_sb", (128, NCOLS), I32))
        gu_sb = E(nc.sbuf_tensor("gu_sb", (128, RGU * WU9), BF16))
        gs_sb = E(nc.sbuf_tensor("gs_sb", (128, NCOLS * 2), BF16))
        t_sb = E(nc.sbuf_tensor("t_sb", (128, RTQ * 48), BF16))
        ident_sb = E(nc.sbuf_tensor("ident_sb", (128, 128), BF16))
        w_sb = E(nc.sbuf_tensor("w_sb", (128, 6 * 128), BF16))
        wcomb_sb = E(nc.sbuf_tensor("wcomb_sb", (12, 128), BF16))
        wc_sb = E(nc.sbuf_tensor("wc_sb", (128, 2), BF16))
        bc_sb = E(nc.sbuf_tensor("bc_sb", (2, 1), F32))
        zbias_sb = E(nc.sbuf_tensor("zbias_sb", (128, 1), F32))
        tab_sb = E(nc.sbuf_tensor("tab_sb", (12, EPAD), BF16))
        x_sb = E(nc.sbuf_tensor("x_sb", (128, 4 * 6 * BW), BF16))
        zr_sb = E(nc.sbuf_tensor("zr_sb", (128, 2 * BW), BF16))
        o_sb = E(nc.sbuf_tensor("o_sb", (2, EPAD), F32))

        ps_t = [E(nc.psum_tensor(f"ps_t{i}", (12, 4 * 128), BF16)) for i in range(2)]
        ps_z = [E(nc.psum_tensor(f"ps_z{i}", (128, BW), F32)) for i in range(4)]
        ps_o = [E(nc.psum_tensor(f"ps_o{i}", (2, BW), F32)) for i in range(2)]

        def quad_cols(q):
            lo = 4 * q * 128
            hi = min((4 * q + 4) * 128, NSTR * 128)
            return lo, hi

        with nc.Block() as block, ExitStack() as sctx:
            SEM = lambda n: sctx.enter_context(nc.semaphore(n))
            srcsemU1 = SEM("srcsemU1")
            srcsemU2 = SEM("srcsemU2")
            srcsemS = SEM("srcsemS")
            wsem = SEM("wsem")
            wsem2 = SEM("wsem2")
            gsemU = [SEM(f"gsemU{i}") for i in range(RGU)]
            gsemS = [SEM(f"gsemS{i}") for i in range(3)]
            tsem = SEM("tsem")      # quad reduces done (1/quad)
            psem = SEM("psem")      # stripe transposes done
            csem = SEM("csem")      # quad copies into tab_sb
            xsem = [SEM(f"xsem{i}") for i in range(4)]
            zsem = SEM("zsem")      # z matmul per block
            rsem = SEM("rsem")      # relu per block
            osem = SEM("osem")      # classifier matmul per block
            ocopA = SEM("ocopA")    # out copy, even blocks (DVE)
            ocopB = SEM("ocopB")    # out copy, odd blocks (Act)
            odmaA = SEM("odmaA")    # even-block stores (SP)
            odmaB = SEM("odmaB")    # odd-block stores (Pool)

            xTv = xT[:].rearrange("(k p) j -> p k j", p=128)

            def _x_load(eng, b):
                if b >= 4:
                    eng.wait_ge(zsem, b - 3)
                eng.dma_start(
                    out=x_sb[:, (b % 4) * 6 * BW:(b % 4 + 1) * 6 * BW]
                        .rearrange("p (k j) -> p k j", k=6),
                    in_=xTv[:, :, b * BW:(b + 1) * BW],
                ).then_inc(xsem[b % 4], 16)

            def _ocopy_act(sc, o):
                sc.wait_ge(osem, o + 1)
                sc.activation(
                    out=o_sb[:, o * BW:(o + 1) * BW],
                    in_=ps_o[o % 2][:],
                    func=mybir.ActivationFunctionType.Identity,
                    bias=bc_sb[:],
                ).then_inc(ocopB, 1)

            def _ocopy_dve(ve, o):
                ve.wait_ge(osem, o + 1)
                ve.tensor_tensor(
                    out=o_sb[:, o * BW:(o + 1) * BW],
                    in0=ps_o[o % 2][:],
                    in1=bc_sb[:].to_broadcast([2, BW]),
                    op=mybir.AluOpType.add,
                ).then_inc(ocopA, 1)

            def _relu(sc, r):
                sc.wait_ge(zsem, r + 1)
                if r >= 2:
                    sc.wait_ge(osem, r - 1)
                sc.activation(
                    out=zr_sb[:, (r % 2) * BW:(r % 2 + 1) * BW],
                    in_=ps_z[r % 4][:],
                    func=mybir.ActivationFunctionType.Relu,
                    bias=zbias_sb[:],
                ).then_inc(rsem, 1)

            @block.sync
            def _(sy):
                cu0 = int(ou[1])
                sy.dma_start(out=srcu_sb[:, 0:cu0],
                             in_=srcu[:, 0:cu0]).then_inc(srcsemU1, 16)
                sy.dma_start(out=srcs_sb[:], in_=srcs[:]).then_inc(srcsemS, 16)
                sy.dma_start(out=srcu_sb[:, cu0:NCOLU],
                             in_=srcu[:, cu0:NCOLU]).then_inc(srcsemU2, 16)
                sy.dma_start(out=ident_sb[:], in_=ident[:]).then_inc(wsem, 16)
                sy.dma_start(out=wcomb_sb[:], in_=wcomb[:]).then_inc(wsem, 16)
                sy.dma_start(out=wc_sb[:], in_=wc[:]).then_inc(wsem, 16)
                sy.dma_start(out=bc_sb[:], in_=bc[:]).then_inc(wsem2, 16)
                sy.dma_start(out=zbias_sb[:], in_=zbias[:]).then_inc(wsem2, 16)
                for k in range(6):
                    sy.dma_start(
                        out=w_sb[:, k * 128:(k + 1) * 128],
                        in_=wer[k * 128:(k + 1) * 128, :],
                    ).then_inc(wsem, 16)
                # even x blocks, interleaved with even-block output stores
                for b in range(NBLK):
                    if b % 2 == 0:
                        _x_load(sy, b)
                    if b >= 6 and (b - 6) % 2 == 0:
                        s_ = b - 6
                        sy.wait_ge(ocopA, s_ // 2 + 1)
                        sy.dma_start(
                            out=out[:, s_ * BW:(s_ + 1) * BW],
                            in_=o_sb[:, s_ * BW:(s_ + 1) * BW],
                        ).then_inc(odmaA, 16)
                for s_ in (20, 22, 24):
                    sy.wait_ge(ocopA, s_ // 2 + 1)
                    sy.dma_start(
                        out=out[:, s_ * BW:(s_ + 1) * BW],
                        in_=o_sb[:, s_ * BW:(s_ + 1) * BW],
                    ).then_inc(odmaA, 16)
                sy.wait_ge(odmaA, 16 * 13)

            @block.scalar
            def _(sc):
                sc.wait_ge(wsem2, 32)
                # odd x blocks + relu(+bias) + odd output copies
                for b in range(NBLK):
                    if b % 2 == 1:
                        _x_load(sc, b)
                    if b >= 1:
                        _relu(sc, b - 1)
                    if b >= 3 and b % 2 == 1:
                        _ocopy_act(sc, b - 2)
                _relu(sc, NBLK - 1)
                _ocopy_act(sc, NBLK - 2)

            def _se_gather(gp, gj):
                q0, q1 = segroups[gj]
                if gj == 0:
                    gp.wait_ge(srcsemS, 16)
                if gj >= 3:
                    gp.wait_ge(tsem, segroups[gj - 3][1])
                d0, d1 = int(os_[q0]), int(os_[q1])
                gp.indirect_dma_start(
                    out=gs_sb[:, d0 * 2:d1 * 2],
                    out_offset=None,
                    in_=tabs[:],
                    in_offset=IndirectOffsetOnAxis(
                        ap=srcs_sb[:, d0:d1], axis=0
                    ),
                ).then_inc(gsemS[gj % 3], 16)

            @block.gpsimd
            def _(gp):
                gp.wait_ge(srcsemU1, 16)
                for q in range(NQ):
                    if q == 1:
                        gp.wait_ge(srcsemU2, 16)
                    if q >= RGU:
                        gp.wait_ge(tsem, q - RGU + 1)
                    c0, c1 = int(ou[q]), int(ou[q + 1])
                    gp.indirect_dma_start(
                        out=gu_sb[:, (q % RGU) * WU9:
                                  (q % RGU) * WU9 + (c1 - c0) * 9],
                        out_offset=None,
                        in_=tabu[:],
                        in_offset=IndirectOffsetOnAxis(
                            ap=srcu_sb[:, c0:c1], axis=0
                        ),
                    ).then_inc(gsemU[q % RGU], 16)
                    # emit se gathers at their first-quad positions
                    for gj, (q0, q1) in enumerate(segroups):
                        if q0 == q:
                            _se_gather(gp, gj)
                # odd-block output stores (late phase; Pool queue is idle)
                for b in range(1, NBLK, 2):
                    gp.wait_ge(ocopB, (b + 1) // 2)
                    gp.dma_start(
                        out=out[:, b * BW:(b + 1) * BW],
                        in_=o_sb[:, b * BW:(b + 1) * BW],
                    ).then_inc(odmaB, 16)
                gp.wait_ge(odmaB, 16 * 12)

            @block.vector
            def _(ve):
                # zero col 11 of every t_sb slot (never written by reduces)
                ve.memset(t_sb[:].rearrange("p (s j) -> p s j", j=12)[:, :, 11:12],
                          0.0)
                ve.memset(tab_sb[:, NSTR * 128:EPAD], 0.0)
                with nc.allow_low_precision(reason="bf16 edge aggregation"):
                    for q in range(NQ):
                        K = _Kq(q)
                        gj = int(sgrp_of[q])
                        ve.wait_ge(gsemU[q % RGU], 16 * (q // RGU + 1))
                        ve.wait_ge(gsemS[gj % 3], 16 * (gj // 3 + 1))
                        if q >= RTQ:
                            qq = q - RTQ
                            ve.wait_ge(psem, 4 * qq + _Kq(qq))
                        tq = t_sb[:, (q % RTQ) * 48:(q % RTQ) * 48 + K * 12] \
                            .rearrange("p (K j) -> p K j", j=12)
                        muq = int(mu[q])
                        ve.tensor_reduce(
                            out=tq[:, :, 0:9],
                            in_=gu_sb[:, (q % RGU) * WU9:
                                      (q % RGU) * WU9 + K * muq * 9]
                                .rearrange("p (K m j) -> p K j m", j=9, m=muq),
                            axis=mybir.AxisListType.X,
                            op=mybir.AluOpType.add,
                        )
                        msq = int(ms[q])
                        d0 = int(os_[q])
                        ve.tensor_reduce(
                            out=tq[:, :, 9:11],
                            in_=gs_sb[:, d0 * 2:d0 * 2 + K * msq * 2]
                                .rearrange("p (K m j) -> p K j m", j=2, m=msq),
                            axis=mybir.AxisListType.X,
                            op=mybir.AluOpType.add,
                        ).then_inc(tsem, 1)
                        # lagged quad copy + even out copies
                        if q >= 1:
                            qq = q - 1
                            lo, hi = quad_cols(qq)
                            ve.wait_ge(psem, 4 * qq + (hi - lo) // 128)
                            ve.tensor_copy(
                                out=tab_sb[:, lo:hi],
                                in_=ps_t[qq % 2][:, 0:hi - lo],
                            ).then_inc(csem, 1)
                            if qq >= 3 and (qq - 3) % 2 == 0:
                                _ocopy_dve(ve, qq - 3)
                lo, hi = quad_cols(NQ - 1)
                ve.wait_ge(psem, 4 * (NQ - 1) + (hi - lo) // 128)
                ve.tensor_copy(
                    out=tab_sb[:, lo:hi],
                    in_=ps_t[(NQ - 1) % 2][:, 0:hi - lo],
                ).then_inc(csem, 1)
                for o in (22, 24):
                    _ocopy_dve(ve, o)

            def _pe_z(te, b):
                te.wait_ge(csem, b + 1)
                te.wait_ge(xsem[b % 4], 16 * (b // 4 + 1))
                if b >= 4:
                    te.wait_ge(rsem, b - 3)
                for k in range(6):
                    te.matmul(
                        ps_z[b % 4][:],
                        w_sb[:, k * 128:(k + 1) * 128],
                        x_sb[:, (b % 4) * 6 * BW + k * BW:
                             (b % 4) * 6 * BW + (k + 1) * BW],
                        start=(k == 0),
                        stop=False,
                    )
                te.matmul(
                    ps_z[b % 4][:],
                    wcomb_sb[:],
                    tab_sb[:, b * BW:(b + 1) * BW],
                    start=False,
                    stop=True,
                ).then_inc(zsem, 1)

            def _pe_cls(te, b):
                te.wait_ge(rsem, b + 1)
                if b >= 2:
                    if b % 2 == 0:
                        te.wait_ge(ocopA, b // 2)
                    else:
                        te.wait_ge(ocopB, (b - 1) // 2)
                te.matmul(
                    ps_o[b % 2][:],
                    wc_sb[:],
                    zr_sb[:, (b % 2) * BW:(b % 2 + 1) * BW],
                    start=True,
                    stop=True,
                ).then_inc(osem, 1)

            def _pe_block(te, b):
                _pe_z(te, b)
                if b >= 1:
                    _pe_cls(te, b - 1)

            @block.tensor
            def _(te):
                te.wait_ge(wsem, 16 * 9)
                for s in range(NSTR):
                    q = s // 4
                    te.wait_ge(tsem, q + 1)
                    if q >= 2:
                        te.wait_ge(csem, q - 1)
                    te.matmul(
                        ps_t[q % 2][:, (s % 4) * 128:(s % 4 + 1) * 128],
                        t_sb[:, (q % RTQ) * 48 + (s % 4) * 12:
                             (q % RTQ) * 48 + (s % 4 + 1) * 12],
                        ident_sb[:],
                        is_transpose=True,
                    ).then_inc(psem, 1)
                    if s % 4 == 3 and s >= 11:
                        _pe_block(te, s // 4 - 2)
                for b in range(NQ - 3, NQ):
                    _pe_block(te, b)
                _pe_cls(te, NQ - 1)

    nc.compile()
    _prog_cache[key] = nc
    _prog_cache["nc"] = nc
    return nc


def _edge_layout(dst, src, rank):
    """Slot layout for one edge type: (core, part, stripe, k_within, src)."""
    key = rank[dst]
    order = np.argsort(key, kind="stable")
    ks = key[order]
    ss = src[order]
    starts = np.searchsorted(ks, np.arange(N_EMAIL))
    k_within = np.arange(dst.shape[0]) - starts[ks]
    core = (ks % NCORE).astype(np.int64)
    pos = ks // NCORE
    return core, pos % 128, pos // 128, k_within, ss


def _host_prep(inputs):
    f32 = np.float32
    x_email = np.asarray(inputs["x_email"], f32)
    x_url = np.asarray(inputs["x_url"], f32)
    x_sender = np.asarray(inputs["x_sender"], f32)

    # per-type augmented gather tables (last row = zeros for padding)
    tabu = np.zeros((N_URL + 1, 9), f32)
    tabu[:N_URL, 0:8] = x_url
    tabu[:N_URL, 8] = 1.0
    tabs = np.zeros((N_SENDER + 1, 2), f32)
    tabs[:N_SENDER, 0] = x_sender[:, 0]
    tabs[:N_SENDER, 1] = 1.0

    # folded weights
    wroot = inputs["Wroot_ue"] + inputs["Wroot_se"]
    wer = np.ascontiguousarray((inputs["W_email"] @ wroot).astype(f32))
    wcomb = np.zeros((12, 128), f32)
    wcomb[0:8] = inputs["W_url"] @ inputs["Wrel_ue"]
    wcomb[8] = inputs["b_url"] @ inputs["Wrel_ue"]
    wcomb[9] = inputs["W_sender"][0] @ inputs["Wrel_se"]
    wcomb[10] = inputs["b_sender"] @ inputs["Wrel_se"]
    zbias = (inputs["brel_ue"] + inputs["brel_se"]
             + inputs["b_email"] @ wroot).astype(f32).reshape(128, 1)

    # ---- degree-sorted layout -------------------------------------------
    dst_ue = np.asarray(inputs["dst_ue"], np.int64)
    dst_se = np.asarray(inputs["dst_se"], np.int64)
    due = np.bincount(dst_ue, minlength=N_EMAIL)
    dse = np.bincount(dst_se, minlength=N_EMAIL)
    perm = np.argsort(-(9 * due + 2 * dse), kind="stable")
    rank = np.empty(N_EMAIL, np.int64)
    rank[perm] = np.arange(N_EMAIL)

    # per-quad uniform slot counts (max over all cores & quad stripes)
    def quad_m(deg):
        d = np.zeros(NQ * 4 * 128 * NCORE, np.int64)
        d[:N_EMAIL] = deg[perm]
        d = d.reshape(NQ, -1)  # 4 stripes x 128 x 8 cores per quad
        return np.maximum(d.max(axis=1), 1)

    mu = quad_m(due)
    ms = quad_m(dse)
    Kq = np.array([_Kq(q) for q in range(NQ)], np.int64)
    ou = np.zeros(NQ + 1, np.int64)
    ou[1:] = np.cumsum(Kq * mu)
    os_ = np.zeros(NQ + 1, np.int64)
    os_[1:] = np.cumsum(Kq * ms)
    NCOLU, NCOLS = int(ou[-1]), int(os_[-1])

    # se gather groups: consecutive quads, <= 512 columns each
    segroups = []
    q = 0
    while q < NQ:
        q1 = q + 1
        while q1 < NQ and os_[q1 + 1] - os_[q] <= 512:
            q1 += 1
        segroups.append((q, int(q1)))
        q = int(q1)

    layout = {"mu": mu, "ms": ms, "ou": ou, "os": os_, "segroups": segroups}
    _prog_cache["layout"] = layout

    src_ue = np.asarray(inputs["src_ue"], np.int64).astype(np.int32)
    src_se = np.asarray(inputs["src_se"], np.int64).astype(np.int32)

    SRCU = np.full((NCORE, 128, NCOLU), ZU, np.int32)
    core, part, stripe, kw, ss = _edge_layout(dst_ue, src_ue, rank)
    qq = stripe // 4
    SRCU[core, part, ou[qq] + (stripe % 4) * mu[qq] + kw] = ss

    SRCS = np.full((NCORE, 128, NCOLS), ZS, np.int32)
    core, part, stripe, kw, ss = _edge_layout(dst_se, src_se, rank)
    qq = stripe // 4
    SRCS[core, part, os_[qq] + (stripe % 4) * ms[qq] + kw] = ss

    identity = np.eye(128, dtype=f32).astype(BF)
    wer_bf = wer.astype(BF)
    wcomb_bf = wcomb.astype(BF)
    wc_bf = np.ascontiguousarray(np.asarray(inputs["Wc"], f32)).astype(BF)
    bc_np = np.asarray(inputs["bc"], f32).reshape(2, 1)
    tabu_bf = tabu.astype(BF)
    tabs_bf = tabs.astype(BF)

    in_maps = []
    for c in range(NCORE):
        xTc = np.zeros((768, EPAD), f32)
        xTc[:, :EPC] = x_email[perm[c::NCORE]].T
        in_maps.append({
            "tabu": tabu_bf,
            "tabs": tabs_bf,
            "srcu": np.ascontiguousarray(SRCU[c]),
            "srcs": np.ascontiguousarray(SRCS[c]),
            "xT": xTc.astype(BF),
            "wer": wer_bf,
            "wcomb": wcomb_bf,
            "wc": wc_bf,
            "bc": bc_np,
            "zbias": zbias,
            "ident": identity,
        })
    _prog_cache["perm"] = perm
    return in_maps


def kernel(**inputs):
    in_maps = _host_prep(inputs)
    nc = _build_program(_prog_cache["layout"])
    perm = _prog_cache["perm"]
    res = None
    last_exc = None
    for _attempt in range(3):
        try:
            res = run_bass_kernel_spmd(nc, in_maps, list(range(NCORE)))
            break
        except Exception as e:  # transient device wedge recovers on retry
            last_exc = e
            import time as _time
            _time.sleep(5.0)
    if res is None:
        raise last_exc
    out = np.empty((N_EMAIL, 2), np.float32)
    for c in range(NCORE):
        out[perm[c::NCORE]] = res.results[c]["out"][:, :EPC].T
    return out
